# revision 22
# baseline (speedup 1.0000x reference)
"""Trainium2 Bass kernel for nn_AntisymmetricLayer — v4 (PE-side reduction).

Same math as kernel.py, but the r-reduction and the lin add run on the
TensorEngine via accumulating matmuls against a 0/1 selection matrix, so the
VectorEngine does ONLY the elementwise products.

Layout trick: computation runs transposed. Per 512-token block:
  GpSimd   : z = x1-x2, s = x1+x2 on whole block [128, 512] bf16
  DMA xbar : transpose -> z^T, s^T [d, n-block] bf16
  PE       : A^T_c = P2_c^T @ z^T  [128kr, 512n] (8 chunks of kr), B^T_c same
             outT = W^T-matmul (lin, start) + sum_c sel_c^T @ prod_c (accum)
  ACT      : stage B^T_c PSUM -> SBUF bf16; evacuate outT -> SBUF
  DVE      : prod_c = A^T_c * B^T_c  (one PSUM + one SBUF operand)
  out in DRAM is [K, n_tokens]; host transposes during unshard.

sel_c[p, k] = 1 iff k == c*8 + p//16  (sums groups of 16 kr-partitions)
"""

import numpy as np
import ml_dtypes

import concourse.bass as bass
import concourse.mybir as mybir
import concourse.tile as tile
from concourse import bacc
from concourse.bass import ts
from concourse.bass_utils import run_bass_kernel_spmd

F32 = mybir.dt.float32
BF16 = mybir.dt.bfloat16

D = 128
K = 64
R = 16
KR = K * R  # 1024
NCHUNK = KR // 128  # 8 kr-chunks of 128
SELW = NCHUNK * 32  # 256 (32-wide strips)
CONST_W = 2 * KR + K + SELW + 2 * 256  # p2|q2|wt|sel|[I|I]|[-I|I] packed
N_CORES = 8
OUT_T = True  # DRAM output is [K, n]; host transposes
TILE = 128
CHUNK_TILES = 4     # tokens per block = 512
BLK = TILE * CHUNK_TILES


def build_bass(n_tokens: int = 16384):
    assert n_tokens % BLK == 0
    n_blocks = n_tokens // BLK

    nc = bacc.Bacc(None, target_bir_lowering=False)

    x1 = nc.declare_dram_parameter("x1", [n_tokens, D], F32, isOutput=False)
    x2 = nc.declare_dram_parameter("x2", [n_tokens, D], F32, isOutput=False)
    cw = nc.declare_dram_parameter("cw", [D, CONST_W], BF16, isOutput=False)
    # output stored transposed [K, n]; host transposes after gather
    out = nc.declare_dram_parameter("out", [K, n_tokens], F32, isOutput=True)

    with tile.TileContext(nc) as tc:
        with (
            tc.tile_pool(name="const", bufs=1) as cpool,
            tc.tile_pool(name="xin", bufs=3) as xpool,
            tc.tile_pool(name="zst", bufs=3) as ztpool,
            tc.tile_pool(name="bsp", bufs=4) as bspool,
            tc.tile_pool(name="prods", bufs=6) as ppool,
            tc.tile_pool(name="outs", bufs=3) as opool,
            tc.tile_pool(name="ptr", bufs=1, space="PSUM") as ptr_pool,
            tc.tile_pool(name="pa", bufs=2, space="PSUM") as pa_pool,
            tc.tile_pool(name="pb", bufs=2, space="PSUM") as pb_pool,
            tc.tile_pool(name="po", bufs=2, space="PSUM") as po_pool,
        ):
            cws = cpool.tile([D, CONST_W], BF16)
            nc.sync.dma_start(cws[:], cw[:])
            p2s = cws[:, 0:KR]
            q2s = cws[:, KR : 2 * KR]
            wts = cws[:, 2 * KR : 2 * KR + K]
            sels = cws[:, 2 * KR + K : 2 * KR + K + SELW]
            idpair = cws[:, 2 * KR + K + SELW : 2 * KR + K + SELW + 256]
            idpairn = cws[:, 2 * KR + K + SELW + 256 :]

            x1v = x1.rearrange("(c a p) d -> c p a d", p=TILE, a=CHUNK_TILES)
            x2v = x2.rearrange("(c a p) d -> c p a d", p=TILE, a=CHUNK_TILES)

            prev = None

            def do_tail(zt, st, j):
                # PE: lin first (opens the outT accumulation group),
                # then per-chunk A/B matmuls with sel-reduce skewed behind
                # NOTE: skip_group_check -- the CoreSim zero-region tracker
                # ignores the out base-partition, so the 32-row strip groups
                # false-positive. HW per-element has_written semantics are
                # exact: the full-width lin matmul (start=True) clears the
                # bank and sets bits for all 64 rows; strip matmuls accumulate.
                outp = po_pool.tile([K, BLK], F32, name=f"outp{j}", tag="outp")
                nc.tensor.matmul(outp[:], wts, zt[:], start=True, stop=False,
                                 skip_group_check=True)

                chunks = []  # (a_psum, prod_sb) pending sel-reduce

                def emit_sel(c, a_ps, b_sb):
                    prod = ppool.tile(
                        [128, BLK], BF16, name=f"prod{j}_{c}", tag="prod"
                    )
                    nc.vector.tensor_mul(prod[:], a_ps[:], b_sb[:])
                    # 32-row strip (c%2): consecutive chunks land on different
                    # col-groups and execute concurrently in the PE array
                    strip = outp[32 * (c % 2) : 32 * (c % 2) + 32, :]
                    nc.tensor.matmul(
                        strip,
                        sels[:, c * 32 : (c + 1) * 32],
                        prod[:],
                        start=False,
                        stop=(c >= NCHUNK - 2),
                        skip_group_check=True,
                    )

                for c in range(NCHUNK):
                    a = pa_pool.tile([128, BLK], F32, name=f"a{j}_{c}", tag="A")
                    nc.tensor.matmul(
                        a[:], p2s[:, ts(c, 128)], zt[:], start=True, stop=True
                    )
                    b = pb_pool.tile([128, BLK], F32, name=f"b{j}_{c}", tag="B")
                    nc.tensor.matmul(
                        b[:], q2s[:, ts(c, 128)], st[:], start=True, stop=True
                    )
                    bs = bspool.tile([128, BLK], BF16, name=f"bs{j}_{c}", tag="bs")
                    nc.scalar.copy(bs[:], b[:])
                    chunks.append((a, bs))
                    # emit sel-reduces in ADJACENT strip pairs so the two
                    # 32-row col-groups execute concurrently in the array
                    if c % 2 == 1:
                        emit_sel(c - 1, *chunks[c - 1])
                        emit_sel(c, *chunks[c])

                # ACT: evacuate outT, then DMA [K, 512] f32 (2KB rows)
                osb = opool.tile([K, BLK], F32, name=f"osb{j}", tag="osb")
                nc.scalar.copy(osb[:], outp[:])
                nc.sync.dma_start(out[:, ts(j, BLK)], osb[:])

            for j in range(n_blocks):
                x1c = xpool.tile([TILE, CHUNK_TILES, D], BF16, name=f"x1c{j}", tag="x1c")
                nc.gpsimd.dma_start(x1c[:], x1v[j])
                x2c = xpool.tile([TILE, CHUNK_TILES, D], BF16, name=f"x2c{j}", tag="x2c")
                nc.gpsimd.dma_start(x2c[:], x2v[j])

                # PE: z^T/s^T via paired transposing matmuls: stationary
                # x1_t streams [I|I] (writes x1^T to both pz_t and ps_t),
                # then x2_t streams [-I|I] accumulating -> pz_t|ps_t.
                # Layout [D, t, (pz|ps)]: 2 subtile-pairs per PSUM bank,
                # accumulation groups run sequentially per bank.
                pzs = ptr_pool.tile([D, 2 * BLK], F32, name=f"pzs{j}", tag="pzs")
                pzv = pzs.rearrange("p (t w) -> p t w", w=2 * TILE)
                for t in range(CHUNK_TILES):
                    pair = pzv[:, t, :]
                    nc.tensor.matmul(pair, x1c[:, t, :], idpair,
                                     start=True, stop=False)
                    nc.tensor.matmul(pair, x2c[:, t, :], idpairn,
                                     start=False, stop=True)

                # evacuate: zt on ACT, st on DVE (strided gather of the
                # per-t halves; inner 128 contiguous)
                zt = ztpool.tile([D, BLK], BF16, name=f"zt{j}", tag="zt")
                nc.scalar.copy(
                    zt.rearrange("p (t w) -> p t w", w=TILE),
                    pzv[:, :, 0:TILE],
                )
                st = ztpool.tile([D, BLK], BF16, name=f"st{j}", tag="st")
                nc.vector.tensor_copy(
                    st.rearrange("p (t w) -> p t w", w=TILE),
                    pzv[:, :, TILE : 2 * TILE],
                )

                if prev is not None:
                    do_tail(*prev)
                prev = (zt, st, j)

            do_tail(*prev)

    nc.finalize()
    return nc


def _perm():
    # out-row for k = 8c+t is  newk = 32*(c%2) + 8*(c//2) + t
    perm = np.zeros(K, dtype=np.int64)
    for c in range(NCHUNK):
        for t in range(8):
            perm[8 * c + t] = 32 * (c % 2) + 8 * (c // 2) + t
    return perm


def _make_sel():
    # sel_c maps kr-partition p to strip-local row 8*(c//2) + p//16
    sel = np.zeros((NCHUNK, 128, 32), dtype=np.float32)
    for c in range(NCHUNK):
        for p in range(128):
            sel[c, p, 8 * (c // 2) + p // 16] = 1.0
    return sel.transpose(1, 0, 2).reshape(128, NCHUNK * 32)


def _shard_and_pack(x1, x2, W_lin, P, Q):
    p2 = P.transpose(1, 0, 2).reshape(D, KR)
    q2 = Q.transpose(1, 0, 2).reshape(D, KR)
    wt = np.ascontiguousarray(W_lin.T)[:, np.argsort(_perm())]
    idp = np.eye(D, dtype=np.float32)
    idpair = np.concatenate([idp, idp], axis=1)
    idpairn = np.concatenate([-idp, idp], axis=1)
    cwv = np.concatenate([p2, q2, wt, _make_sel(), idpair, idpairn], axis=1).astype(
        ml_dtypes.bfloat16
    )
    assert cwv.shape == (D, CONST_W)

    in_maps = []
    for b in range(N_CORES):
        in_maps.append(
            {
                "x1": np.ascontiguousarray(x1[b]),
                "x2": np.ascontiguousarray(x2[b]),
                "cw": cwv,
            }
        )
    return in_maps


def postprocess(out_raw):
    """Per-core raw DRAM output [K, n] (permuted rows) -> [n, K] natural."""
    return np.ascontiguousarray(out_raw[_perm(), :].T)


def kernel(x1, x2, W_lin, P, Q):
    assert x1.shape == (N_CORES, 16384, D) and x2.shape == x1.shape
    nc = build_bass(16384)
    in_maps = _shard_and_pack(x1, x2, W_lin, P, Q)
    res = run_bass_kernel_spmd(nc, in_maps, core_ids=list(range(N_CORES)))
    out = np.stack(
        [postprocess(res.results[b]["out"]) for b in range(N_CORES)], axis=0
    )
    return out.astype(np.float32)
